# revision 1
# baseline (speedup 1.0000x reference)
"""Trainium2 Bass kernel: causal sliding-window GQA self-attention.

Problem: B=2, T=2048, C=2048, 16 q-heads / 4 kv-heads, head_dim=128,
RoPE, sliding window 512, projections Wq/Wk/Wv/Wo.

Sharding: 8 cores = DP(batch=2) x TP(head-groups=4).  Core c handles
batch c//4 and q-heads [4*(c%4), 4*(c%4)+4) (one kv head c%4).  Each
core computes a partial output contribution [T, C]; the host sums the
4 head-group partials per batch.

Per-core kernel (all matmuls bf16, f32 accumulation):
  - host passes x[b]^T so the contraction dim (C) lands on partitions
  - Q^T/K^T [hd, t] via PE matmul, RoPE applied during PSUM eviction
    (scale 1/sqrt(hd) folded into the Q rope tables; the rotate-half is
    a cross-partition DVE multiply against sign-folded sin tables, and
    the final add runs on GpSimd to keep DVE off the critical path)
  - V^T computed, then PE-transposed to V [t, hd]
  - per (head, 128-query block): scores S^T [j, q] for the <=5 key
    blocks of the 640-wide causal window (off-diagonal blocks in a
    one-bank [128,512] PSUM tile, the diagonal block riding the "acc"
    tag rotation -- the whole score set costs 3 banks less than a
    naive [128,640]x2, bought back as deeper PV pipelining), exp on ScalarE
    (no max subtraction -- max |score| measured 5.5 on this input
    distribution, exp stays tiny vs f32/bf16 range), 0/1 band-mask
    multiply (only the two edge key-blocks are not all-ones), PV
    matmuls as one PSUM accumulation group.  Softmax denominators stay
    off the PE entirely: the P^T tiles are pairwise-added on DVE (bf16
    fast mode) and one GpSimd partition_all_reduce produces the
    per-query sums already broadcast across every partition; a bf16
    reciprocal then feeds the y^T eviction multiply (this sidesteps the
    partition-vs-free-dim mismatch that otherwise forces transposes).
    Per-head Wo matmuls accumulate over the 4 heads in PSUM.  DMA
    emission order is tuned so the shared 360GB/s DMA path stays ahead
    of the PE during the projection phase.

The Wo matmuls for query block qb are emitted one iteration late
(during attention of qb+1): they are then always ready-to-execute when
the static scheduler places them, giving it dense PE work to slot into
the attention chains' wait windows.

Timeline-sim (cost model) per-core exec: ~200.2us (PE busy ~170us, at
the warm bf16 roofline for this instruction mix -- the residual ~39us
is the xT-stream-gated start, per-iteration pipeline wiggle against
8 fully-committed PSUM banks, and the drain tail); rel err vs the f32
reference ~4.9e-3 (bf16 quantization of inputs/intermediates/partials).
"""

import os
import sys

for _p in ("/opt/trn_rl_repo", "/root/.axon_site/_ro/trn_rl_repo"):
    if os.path.isdir(_p) and _p not in sys.path:
        sys.path.append(_p)

import numpy as np
import ml_dtypes

BF16 = ml_dtypes.bfloat16

B, T, C = 2, 2048, 2048
H, KVH, HD = 16, 4, 128
WIN = 512
ROPE_BASE = 10000.0
NCORES = 8
TPG = 4           # tensor-parallel group count (head groups)
HPG = H // TPG    # q-heads per core
SCALE = 1.0 / float(np.sqrt(np.float32(HD)))
NWINB = WIN // 128 + 1   # 5 key blocks cover the 640-wide window

_NC_CACHE = {}


def _rope_tables(t_len):
    # Match reference: angles computed in float32.
    inv = (1.0 / (np.float32(ROPE_BASE) ** (np.arange(0, HD, 2, dtype=np.float32) / np.float32(HD)))).astype(np.float32)
    ang = np.arange(t_len, dtype=np.float32)[None, :] * inv[:, None]   # [64, T]
    cosT = np.concatenate([np.cos(ang), np.cos(ang)], axis=0)          # [128, T]
    sinT = np.sin(ang)
    sin_swap = np.concatenate([-sinT, sinT], axis=0)                   # [128, T]
    return cosT.astype(np.float32), sin_swap.astype(np.float32)


def _band_maskT():
    # maskT[c, r] = 1 iff query row r may attend key col c of the
    # 640-wide window (c = j - (qs - 512)):  r+1 <= c <= r+512.
    r = np.arange(128)[None, :]
    c = np.arange(NWINB * 128)[:, None]
    return ((r + 1 <= c) & (c <= r + WIN)).astype(np.float32)          # [640, 128]


def build_nc(t_len=T):
    """Build + compile the per-core Bass module (SPMD, identical on all cores)."""
    import concourse.mybir as mybir
    import concourse.tile as tile
    from concourse import bacc
    from concourse import bass_isa

    dt = mybir.dt
    NQB = t_len // 128        # query/key blocks
    NCB = C // 128            # contraction blocks for projections
    NTB = t_len // 512        # 512-wide t-blocks for projections

    nc = bacc.Bacc("TRN2", target_bir_lowering=False, debug=False, num_devices=NCORES)

    def din(name, shape, d=dt.bfloat16):
        return nc.dram_tensor(name, shape, d, kind="ExternalInput").ap()

    xT_d = din("xT", [C, t_len])
    wq_d = din("wq", [C, HPG * HD])
    wk_d = din("wk", [C, HD])
    wv_d = din("wv", [C, HD])
    wo_d = din("wo", [HPG * HD, C])
    cosq_d = din("cosq", [HD, t_len])
    sinq_d = din("sinq", [HD, t_len])
    cosk_d = din("cosk", [HD, t_len])
    sink_d = din("sink", [HD, t_len])
    maskT_d = din("maskT", [NWINB * 128, 128])
    ident_d = din("ident", [128, 128])
    out_d = nc.dram_tensor("out", [t_len, C], dt.bfloat16, kind="ExternalOutput").ap()

    with tile.TileContext(nc) as tc:
        with tc.tile_pool(name="persist", bufs=1) as pp:
            xT_sb = pp.tile([128, NCB * t_len], dt.bfloat16, tag="xT")
            wq_sb = pp.tile([128, NCB * HPG * HD], dt.bfloat16, tag="wq")
            wk_sb = pp.tile([128, NCB * HD], dt.bfloat16, tag="wk")
            wv_sb = pp.tile([128, NCB * HD], dt.bfloat16, tag="wv")
            wo_sb = pp.tile([128, HPG * C], dt.bfloat16, tag="wo")
            QT_sb = [[pp.tile([128, 512], dt.bfloat16, tag=f"QT{h}_{tb}", name=f"QT{h}_{tb}")
                      for tb in range(NTB)] for h in range(HPG)]
            KT_sb = pp.tile([128, t_len], dt.bfloat16, tag="KT")
            VT_sb = pp.tile([128, t_len], dt.bfloat16, tag="VT")
            V_sb = pp.tile([128, t_len], dt.bfloat16, tag="V")
            cosq_sb = pp.tile([128, t_len], dt.bfloat16, tag="cosq")
            sinq_sb = pp.tile([128, t_len], dt.bfloat16, tag="sinq")
            cosk_sb = pp.tile([128, t_len], dt.bfloat16, tag="cosk")
            sink_sb = pp.tile([128, t_len], dt.bfloat16, tag="sink")
            maskT_sb = pp.tile([128, NWINB * 128], dt.bfloat16, tag="maskT")
            ident_sb = pp.tile([128, 128], dt.bfloat16, tag="ident")

            # Load order matters: the shared DMA path is the projection-phase
            # rate limiter.  Small constants + rope tables + V/K weights first
            # (first consumers), then xT/wq interleaved, wo (attention-only) last.
            # Weights load as single rearranged-AP DMAs: each separate small
            # DMA costs ~625ns HWDGE + 900ns sem propagation, which dominates
            # its payload and throttles the xT stream the PE is waiting on.
            for cb in range(NCB):
                if cb % 4 == 0:
                    # wv/wk in 4-cb quads: fewer HWDGE slots + sem props on the
                    # stream the projection matmuls are gated by.  wv precedes
                    # its xT tile (V is the first consumer); wk trails by one
                    # tile (K MMs interleave one group behind V).
                    nc.sync.dma_start(
                        wv_sb[:, cb * HD:(cb + 4) * HD].rearrange("p (c h) -> p c h", h=HD),
                        wv_d[cb * 128:(cb + 4) * 128, :].rearrange("(c p) h -> p c h", p=128))
                nc.sync.dma_start(xT_sb[:, cb * t_len:(cb + 1) * t_len], xT_d[cb * 128:(cb + 1) * 128, :])
                if cb % 4 == 1:
                    nc.sync.dma_start(
                        wk_sb[:, (cb - 1) * HD:(cb + 3) * HD].rearrange("p (c h) -> p c h", h=HD),
                        wk_d[(cb - 1) * 128:(cb + 3) * 128, :].rearrange("(c p) h -> p c h", p=128))
            nc.sync.dma_start(cosk_sb[:], cosk_d[:])
            nc.sync.dma_start(sink_sb[:], sink_d[:])
            nc.sync.dma_start(ident_sb[:], ident_d[:])
            for cb in range(NCB):
                nc.sync.dma_start(wq_sb[:, cb * HPG * HD:(cb + 1) * HPG * HD], wq_d[cb * 128:(cb + 1) * 128, :])
                if cb == 3:
                    nc.sync.dma_start(cosq_sb[:], cosq_d[:])
                    nc.sync.dma_start(sinq_sb[:], sinq_d[:])
            nc.sync.dma_start(maskT_sb[:].rearrange("p (m c) -> p m c", c=128),
                              maskT_d[:].rearrange("(m p) c -> p m c", p=128))
            nc.sync.dma_start(wo_sb[:].rearrange("p (h c) -> p h c", c=C),
                              wo_d[:].rearrange("(h p) c -> p h c", p=128))

            # ---------------- projections ----------------
            with tc.tile_pool(name="proj_ps", bufs=7, space="PSUM") as pps, \
                 tc.tile_pool(name="rope_scr", bufs=4) as rsc:

                def rope_evict(ps, dst, cos_sb, sin_sb, tb):
                    sl = slice(tb * 512, (tb + 1) * 512)
                    t1 = rsc.tile([128, 512], dt.float32, tag="t1")
                    t2 = rsc.tile([128, 512], dt.float32, tag="t2")
                    nc.vector.tensor_mul(t1[:], ps[:], cos_sb[:, sl])
                    nc.vector.tensor_mul(t2[0:64, :], ps[64:128, :], sin_sb[0:64, sl])
                    nc.vector.tensor_mul(t2[64:128, :], ps[0:64, :], sin_sb[64:128, sl])
                    nc.gpsimd.tensor_add(dst, t1[:], t2[:])

                for tb in range(NTB):
                    ps = pps.tile([128, 512], dt.float32, tag="ps", name="ps")
                    for cb in range(NCB):
                        nc.tensor.matmul(
                            ps[:], wv_sb[:, cb * HD:(cb + 1) * HD],
                            xT_sb[:, cb * t_len + tb * 512: cb * t_len + (tb + 1) * 512],
                            start=(cb == 0), stop=(cb == NCB - 1))
                    nc.any.tensor_copy(VT_sb[:, tb * 512:(tb + 1) * 512], ps[:])
                    ps = pps.tile([128, 512], dt.float32, tag="ps", name="ps")
                    for cb in range(NCB):
                        nc.tensor.matmul(
                            ps[:], wk_sb[:, cb * HD:(cb + 1) * HD],
                            xT_sb[:, cb * t_len + tb * 512: cb * t_len + (tb + 1) * 512],
                            start=(cb == 0), stop=(cb == NCB - 1))
                    rope_evict(ps, KT_sb[:, tb * 512:(tb + 1) * 512], cosk_sb, sink_sb, tb)
                with tc.tile_pool(name="tr_ps", bufs=1, space="PSUM") as tps:
                    for jb in range(NQB):
                        tp = tps.tile([128, 128], dt.bfloat16, tag="tp")
                        nc.tensor.transpose(tp[:], VT_sb[:, jb * 128:(jb + 1) * 128], ident_sb[:])
                        nc.any.tensor_copy(V_sb[:, jb * 128:(jb + 1) * 128], tp[:])
                for tb in range(NTB):
                    for h in range(HPG):
                        ps = pps.tile([128, 512], dt.float32, tag="ps")
                        for cb in range(NCB):
                            nc.tensor.matmul(
                                ps[:],
                                wq_sb[:, cb * HPG * HD + h * HD: cb * HPG * HD + (h + 1) * HD],
                                xT_sb[:, cb * t_len + tb * 512: cb * t_len + (tb + 1) * 512],
                                start=(cb == 0), stop=(cb == NCB - 1))
                        rope_evict(ps, QT_sb[h][tb][:], cosq_sb, sinq_sb, tb)

            # ---------------- attention + Wo ----------------
            with tc.tile_pool(name="st_ps", bufs=3, space="PSUM") as stp, \
                 tc.tile_pool(name="acc_ps", bufs=3, space="PSUM") as accp, \
                 tc.tile_pool(name="wo_ps", bufs=2, space="PSUM") as wop, \
                 tc.tile_pool(name="attn_sb", bufs=16) as asb, \
                 tc.tile_pool(name="yn_sb", bufs=2) as ysb, \
                 tc.tile_pool(name="out_sb", bufs=2) as osb:
                Exp = mybir.ActivationFunctionType.Exp
                def emit_wo(wo_qb, wo_ynT):
                    ostg = osb.tile([128, C], dt.bfloat16, tag="ostg", name="ostg")
                    for cb4 in range(C // 512):
                        wps = wop.tile([128, 512], dt.float32, tag="wps", name="wps")
                        for hh in range(HPG):
                            nc.tensor.matmul(
                                wps[:], wo_ynT[:, hh * 128:(hh + 1) * 128],
                                wo_sb[:, hh * C + cb4 * 512: hh * C + (cb4 + 1) * 512],
                                start=(hh == 0), stop=(hh == HPG - 1))
                        nc.any.tensor_copy(ostg[:, cb4 * 512:(cb4 + 1) * 512], wps[:])
                        if wo_qb >= NQB - 2:
                            nc.sync.dma_start(
                                out_d[wo_qb * 128:(wo_qb + 1) * 128, cb4 * 512:(cb4 + 1) * 512],
                                ostg[:, cb4 * 512:(cb4 + 1) * 512])
                    if wo_qb < NQB - 2:
                        nc.sync.dma_start(out_d[wo_qb * 128:(wo_qb + 1) * 128, :], ostg[:])

                pend = []
                for qb in range(NQB):
                    nwin = min(qb, NWINB - 1) + 1
                    ynT = ysb.tile([128, HPG * 128], dt.bfloat16, tag="ynT")
                    for h in range(HPG):
                        qt = QT_sb[h][qb // 4]
                        qsl = slice((qb % 4) * 128, (qb % 4 + 1) * 128)
                        # off-diagonal score blocks in a 1-bank tile; the
                        # diagonal block rides the "acc" tag rotation so the
                        # whole score set costs 1 bank less than a [128,640].
                        st = stp.tile([128, 512], dt.float32, tag="st", name="st") if nwin > 1 else None
                        std = accp.tile([128, 128], dt.float32, tag="acc", name="std")
                        for i in range(nwin):
                            jb = qb - nwin + 1 + i
                            out_sl = std[:] if i == nwin - 1 else st[:, i * 128:(i + 1) * 128]
                            nc.tensor.matmul(
                                out_sl,
                                KT_sb[:, jb * 128:(jb + 1) * 128],
                                qt[:, qsl], start=True, stop=True)
                        acc = accp.tile([128, 128], dt.float32, tag="acc")
                        pexp = asb.tile([128, NWINB * 128], dt.bfloat16, tag="pexp")
                        if nwin > 1:
                            nc.scalar.activation(pexp[:, 0:(nwin - 1) * 128],
                                                 st[:, 0:(nwin - 1) * 128], Exp)
                        nc.scalar.activation(pexp[:, (nwin - 1) * 128:nwin * 128], std[:], Exp)
                        pms = []
                        for i in range(nwin):
                            m = i + NWINB - nwin
                            if m == 0 or m == NWINB - 1:
                                pm = asb.tile([128, 128], dt.bfloat16, tag="pmask")
                                nc.vector.tensor_mul(pm[:], pexp[:, i * 128:(i + 1) * 128],
                                                     maskT_sb[:, m * 128:(m + 1) * 128])
                                pms.append(pm[:])
                            else:
                                pms.append(pexp[:, i * 128:(i + 1) * 128])
                        for i in range(nwin):
                            jb = qb - nwin + 1 + i
                            nc.tensor.matmul(acc[:], V_sb[:, jb * 128:(jb + 1) * 128], pms[i],
                                             start=(i == 0), stop=(i == nwin - 1))
                        work = list(pms)
                        while len(work) > 1:
                            nxt = []
                            for a, b in zip(work[0::2], work[1::2]):
                                t = asb.tile([128, 128], dt.bfloat16, tag="padd", name="padd")
                                nc.vector.tensor_add(t[:], a, b)
                                nxt.append(t[:])
                            if len(work) % 2:
                                nxt.append(work[-1])
                            work = nxt
                        sbc = asb.tile([128, 128], dt.float32, tag="sbc")
                        nc.gpsimd.partition_all_reduce(sbc[:], work[0], channels=128,
                                                       reduce_op=bass_isa.ReduceOp.add)
                        rbc = asb.tile([128, 128], dt.bfloat16, tag="rbc")
                        with nc.allow_low_precision("softmax denominator reciprocal; 2e-2 rel-err budget"):
                            nc.vector.reciprocal(rbc[:], sbc[:])
                        nc.vector.tensor_mul(ynT[:, h * 128:(h + 1) * 128], acc[:], rbc[:])
                    pend.append((qb, ynT))
                    if len(pend) > 1:
                        emit_wo(*pend.pop(0))
                while pend:
                    emit_wo(*pend.pop(0))

    nc.compile()
    return nc


def _get_nc(t_len=T):
    if t_len not in _NC_CACHE:
        _NC_CACHE[t_len] = build_nc(t_len)
    return _NC_CACHE[t_len]


def host_inputs(x, Wq, Wk, Wv, Wo, t_len=T):
    """Per-core input shards (8 dicts)."""
    x = np.asarray(x, np.float32)
    Wq = np.asarray(Wq, np.float32)
    Wk = np.asarray(Wk, np.float32)
    Wv = np.asarray(Wv, np.float32)
    Wo = np.asarray(Wo, np.float32)
    cosT, sin_swap = _rope_tables(t_len)
    common = {
        "ident": np.eye(128, dtype=np.float32).astype(BF16),
        "cosq": (cosT * SCALE).astype(BF16),
        "sinq": (sin_swap * SCALE).astype(BF16),
        "cosk": cosT.astype(BF16),
        "sink": sin_swap.astype(BF16),
        "maskT": _band_maskT().astype(BF16),
    }
    in_maps = []
    for core in range(NCORES):
        b, hg = core // TPG, core % TPG
        m = dict(common)
        m["xT"] = np.ascontiguousarray(x[b, :t_len, :].T).astype(BF16)
        m["wq"] = np.ascontiguousarray(Wq[:, hg * HPG * HD:(hg + 1) * HPG * HD]).astype(BF16)
        m["wk"] = np.ascontiguousarray(Wk[:, hg * HD:(hg + 1) * HD]).astype(BF16)
        m["wv"] = np.ascontiguousarray(Wv[:, hg * HD:(hg + 1) * HD]).astype(BF16)
        m["wo"] = np.ascontiguousarray(Wo[hg * HPG * HD:(hg + 1) * HPG * HD, :]).astype(BF16)
        in_maps.append(m)
    return in_maps


def kernel(x, Wq, Wk, Wv, Wo):
    from concourse import bass_utils

    nc = _get_nc(T)
    in_maps = host_inputs(x, Wq, Wk, Wv, Wo, T)
    res = bass_utils.run_bass_kernel_spmd(nc, in_maps, core_ids=list(range(NCORES)))
    out = np.zeros((B, T, C), np.float32)
    for core in range(NCORES):
        out[core // TPG] += res.results[core]["out"].astype(np.float32)
    return out


def core_reference(x_b, Wq, Wk, Wv, Wo, hg, t_len=T):
    """Numpy reference of one core's partial output (f32 math, for dev tests)."""
    xb = np.asarray(x_b, np.float64)[:t_len]
    q = xb @ np.float64(Wq[:, hg * HPG * HD:(hg + 1) * HPG * HD])    # [T, 512]
    k = xb @ np.float64(Wk[:, hg * HD:(hg + 1) * HD])                # [T, 128]
    v = xb @ np.float64(Wv[:, hg * HD:(hg + 1) * HD])
    cosT, sin_swap = _rope_tables(t_len)
    cos = cosT.T.astype(np.float64)
    sinsw = sin_swap.T.astype(np.float64)

    def rope(z):
        zsw = np.concatenate([z[:, HD // 2:], z[:, :HD // 2]], axis=1)
        sgn = np.concatenate([sinsw[:, :HD // 2], sinsw[:, HD // 2:]], axis=1)
        return z * cos + zsw * sgn

    out = np.zeros((t_len, C), np.float64)
    i = np.arange(t_len)[:, None]
    j = np.arange(t_len)[None, :]
    allowed = (j <= i) & (i - j < WIN)
    kr = rope(k)
    for h in range(HPG):
        qh = rope(q[:, h * HD:(h + 1) * HD]) * SCALE
        s = qh @ kr.T
        s = np.where(allowed, s, -np.inf)
        p = np.exp(s - s.max(axis=1, keepdims=True))
        p /= p.sum(axis=1, keepdims=True)
        y = p @ v
        out += y @ np.float64(Wo[hg * HPG * HD + h * HD: hg * HPG * HD + (h + 1) * HD, :])
    return out.astype(np.float32)



# revision 6
# speedup vs baseline: 1.1772x; 1.1772x over previous
"""Trainium2 Bass kernel: causal sliding-window GQA self-attention.

Problem: B=2, T=2048, C=2048, 16 q-heads / 4 kv-heads, head_dim=128,
RoPE, sliding window 512, projections Wq/Wk/Wv/Wo.

Sharding: 8 cores = DP(batch=2) x TP(head-groups=4).  Core c handles
batch c//4 and q-heads [4*(c%4), 4*(c%4)+4) (one kv head c%4).  Each
core computes a partial output contribution [T, C]; the host sums the
4 head-group partials per batch (and divides by the 64^2 weight
pre-scale, see below).

Per-core kernel:
  - Projections and the Wo matmul run in fp8-e4m3 DoubleRow perf mode
    (two contraction rows per PE pass, 0.5 cycles/output-column: 4x the
    bf16 MAC rate).  Full precision is recovered with a 3-term hi-lo
    split: x ~ x_hi + x_lo (fp8 pair, host-prepared), W ~ W_hi + W_lo,
    and x@W ~ xhi@Whi + xlo@Whi + xhi@Wlo -- 24 DoubleRow matmuls per
    2048-contraction tile vs 16 bf16 matmuls, i.e. 0.75x the PE time
    with accuracy slightly better than a bf16 matmul.  Weights are
    pre-scaled by 64 on the host so W values (~0.02) sit in the fp8
    normal range; the 1/64 is folded into the RoPE eviction tables for
    Q/K, carried harmlessly through V->y->Wo for the rest, and divided
    out on the host (out = psum / 4096).
  - Attention stays bf16 (contraction is only 128 there, DoubleRow
    buys nothing at equal accuracy) but batches the 4 GQA q-heads of
    the shared kv-head into one free-dim-512 stream: per (128-query
    block, 128-key block) ONE score matmul [keys, 4*128] and ONE PV
    accumulation into ynT [hd, 4*128], quartering instruction counts.
  - exp on ScalarE (no max subtraction; max |score| ~5.5 on this input
    distribution), band-mask multiplies only on the two edge key
    blocks (leading edge on GpSimd, diagonal on DVE), softmax
    denominators via DVE pairwise adds + one GpSimd
    partition_all_reduce, reciprocal on DVE.
  - ynT is written as an fp8 hi/lo pair (DVE mul + ScalarE cast + DVE
    sub) feeding the 3-term DoubleRow Wo.
  - PE program order per query block: scores(qb) -> Wo(qb-1) -> PV(qb),
    so the late-emitted Wo matmuls fill the exp/mask latency window of
    the current block's attention chain.
  - x streams tb-major (512 t-columns at a time across all 16
    contraction row-blocks) so the first projection tile is gated on
    ~2.5MB of DMA instead of the whole 8MB x load.
"""

import os
import sys

for _p in ("/opt/trn_rl_repo", "/root/.axon_site/_ro/trn_rl_repo"):
    if os.path.isdir(_p) and _p not in sys.path:
        sys.path.append(_p)

import numpy as np
import ml_dtypes

BF16 = ml_dtypes.bfloat16
F8 = ml_dtypes.float8_e4m3fn

B, T, C = 2, 2048, 2048
H, KVH, HD = 16, 4, 128
WIN = 512
ROPE_BASE = 10000.0
NCORES = 8
TPG = 4           # tensor-parallel group count (head groups)
HPG = H // TPG    # q-heads per core
SCALE = 1.0 / float(np.sqrt(np.float32(HD)))
NWINB = WIN // 128 + 1   # 5 key blocks cover the 640-wide window
NCB = C // 128           # contraction row-blocks for projections
WSC = 64.0               # host weight pre-scale (fp8 subnormal dodge)

_NC_CACHE = {}


def _rope_tables(t_len):
    # Match reference: angles computed in float32.
    inv = (1.0 / (np.float32(ROPE_BASE) ** (np.arange(0, HD, 2, dtype=np.float32) / np.float32(HD)))).astype(np.float32)
    ang = np.arange(t_len, dtype=np.float32)[None, :] * inv[:, None]   # [64, T]
    cosT = np.concatenate([np.cos(ang), np.cos(ang)], axis=0)          # [128, T]
    sinT = np.sin(ang)
    sin_swap = np.concatenate([-sinT, sinT], axis=0)                   # [128, T]
    return cosT.astype(np.float32), sin_swap.astype(np.float32)


def _band_maskT4():
    # maskT[c, r] = 1 iff query row r may attend key col c of the
    # 640-wide window (c = j - (qs - 512)):  r+1 <= c <= r+512.
    # Replicated x4 along the free dim for the 4-head-batched layout.
    r = np.arange(128)[None, :]
    c = np.arange(NWINB * 128)[:, None]
    m = ((r + 1 <= c) & (c <= r + WIN)).astype(np.float32)             # [640, 128]
    return np.tile(m, (1, HPG))                                        # [640, 512]


def _split_f8(a):
    """fp8 hi/lo pair: a ~ hi + lo with ~7-bit effective mantissa."""
    a32 = np.asarray(a, np.float32)
    hi = a32.astype(F8)
    lo = (a32 - hi.astype(np.float32)).astype(F8)
    return hi, lo


def build_nc(t_len=T):
    """Build + compile the per-core Bass module (SPMD, identical on all cores)."""
    import concourse.mybir as mybir
    import concourse.tile as tile
    from concourse import bacc
    from concourse import bass_isa

    dt = mybir.dt
    DRow = mybir.MatmulPerfMode.DoubleRow
    NQB = t_len // 128        # query/key blocks
    NTB = t_len // 512        # 512-wide t-blocks for projections

    nc = bacc.Bacc("TRN2", target_bir_lowering=False, debug=False, num_devices=NCORES)

    def din(name, shape, d=dt.float8e4):
        return nc.dram_tensor(name, shape, d, kind="ExternalInput").ap()

    xhi_d = din("xhi", [C, t_len])
    xlo_d = din("xlo", [C, t_len])
    wqhi_d = din("wqhi", [C, HPG * HD])
    wqlo_d = din("wqlo", [C, HPG * HD])
    wkhi_d = din("wkhi", [C, HD])
    wklo_d = din("wklo", [C, HD])
    wvhi_d = din("wvhi", [C, HD])
    wvlo_d = din("wvlo", [C, HD])
    wohi_d = din("wohi", [HPG * HD, C])
    wolo_d = din("wolo", [HPG * HD, C])
    cosq_d = din("cosq", [HD, t_len], dt.bfloat16)
    sinq_d = din("sinq", [HD, t_len], dt.bfloat16)
    cosk_d = din("cosk", [HD, t_len], dt.bfloat16)
    sink_d = din("sink", [HD, t_len], dt.bfloat16)
    maskT4_d = din("maskT4", [NWINB * 128, HPG * 128], dt.bfloat16)
    out_d = nc.dram_tensor("out", [t_len, C], dt.bfloat16, kind="ExternalOutput").ap()

    with tile.TileContext(nc) as tc:
        with tc.tile_pool(name="persist", bufs=1) as pp:
            f8 = dt.float8e4
            xhi_sb = pp.tile([128, NCB * t_len], f8, tag="xhi")
            xlo_sb = pp.tile([128, NCB * t_len], f8, tag="xlo")
            wqhi_sb = pp.tile([128, NCB * HPG * HD], f8, tag="wqhi")
            wqlo_sb = pp.tile([128, NCB * HPG * HD], f8, tag="wqlo")
            wkhi_sb = pp.tile([128, NCB * HD], f8, tag="wkhi")
            wklo_sb = pp.tile([128, NCB * HD], f8, tag="wklo")
            wvhi_sb = pp.tile([128, NCB * HD], f8, tag="wvhi")
            wvlo_sb = pp.tile([128, NCB * HD], f8, tag="wvlo")
            wohi_sb = pp.tile([128, HPG * C], f8, tag="wohi")
            wolo_sb = pp.tile([128, HPG * C], f8, tag="wolo")
            cosq_sb = pp.tile([128, t_len], dt.bfloat16, tag="cosq")
            sinq_sb = pp.tile([128, t_len], dt.bfloat16, tag="sinq")
            cosk_sb = pp.tile([128, t_len], dt.bfloat16, tag="cosk")
            sink_sb = pp.tile([128, t_len], dt.bfloat16, tag="sink")
            maskT4_sb = pp.tile([128, NWINB * HPG * 128], dt.bfloat16, tag="maskT4")
            QT4_sb = pp.tile([128, NQB * HPG * 128], dt.bfloat16, tag="QT4")
            KT_sb = pp.tile([128, t_len], dt.bfloat16, tag="KT")
            V_sb = pp.tile([128, t_len], dt.bfloat16, tag="V")

            # 3D chunk views for DoubleRow operand pairing.
            def xv(ts):
                return ts[:].rearrange("p (c t) -> p c t", t=t_len)

            def wv_(ts, m):
                return ts[:].rearrange("p (c m) -> p c m", m=m)

            # ---- DMA emission order: first consumers first; x streams
            # tb-major so the first projection tile gates on ~2.5MB.
            for ws, wd in ((wvhi_sb, wvhi_d), (wvlo_sb, wvlo_d),
                           (wkhi_sb, wkhi_d), (wklo_sb, wklo_d)):
                nc.sync.dma_start(wv_(ws, HD), wd.rearrange("(c p) h -> p c h", p=128))
            for tb in range(NTB):
                tsl = slice(tb * 512, (tb + 1) * 512)
                for xs, xd in ((xhi_sb, xhi_d), (xlo_sb, xlo_d)):
                    for cb in range(0, NCB, 4):
                        nc.sync.dma_start(
                            xv(xs)[:, cb:cb + 4, tsl],
                            xd[cb * 128:(cb + 4) * 128, tsl].rearrange("(c p) t -> p c t", p=128))
                if tb == 0:
                    nc.sync.dma_start(wv_(wqhi_sb, HPG * HD), wqhi_d.rearrange("(c p) m -> p c m", p=128))
                    nc.sync.dma_start(wv_(wqlo_sb, HPG * HD), wqlo_d.rearrange("(c p) m -> p c m", p=128))
                    nc.sync.dma_start(cosk_sb[:], cosk_d)
                    nc.sync.dma_start(sink_sb[:], sink_d)
                    nc.sync.dma_start(cosq_sb[:], cosq_d)
                    nc.sync.dma_start(sinq_sb[:], sinq_d)
                if tb == 1:
                    nc.sync.dma_start(maskT4_sb[:].rearrange("p (m c) -> p m c", c=HPG * 128),
                                      maskT4_d.rearrange("(m p) c -> p m c", p=128))
                    nc.sync.dma_start(wv_(wohi_sb, C), wohi_d.rearrange("(h p) c -> p h c", p=128))
                    nc.sync.dma_start(wv_(wolo_sb, C), wolo_d.rearrange("(h p) c -> p h c", p=128))

            TERMS_V = ((xhi_sb, wvhi_sb), (xlo_sb, wvhi_sb), (xhi_sb, wvlo_sb))
            TERMS_K = ((xhi_sb, wkhi_sb), (xlo_sb, wkhi_sb), (xhi_sb, wklo_sb))
            TERMS_Q = ((xhi_sb, wqhi_sb), (xlo_sb, wqhi_sb), (xhi_sb, wqlo_sb))

            # ---------------- projections ----------------
            with tc.tile_pool(name="proj_ps", bufs=5, space="PSUM") as pps, \
                 tc.tile_pool(name="v_ps", bufs=2, space="PSUM") as vpp, \
                 tc.tile_pool(name="rope_scr", bufs=3) as rsc:

                def rope_evict(ps, dst, cos_sb, sin_sb, tb, dst3=None):
                    sl = slice(tb * 512, (tb + 1) * 512)
                    t1 = rsc.tile([128, 512], dt.float32, tag="t1")
                    t2 = rsc.tile([128, 512], dt.float32, tag="t2")
                    nc.vector.tensor_mul(t1[:], ps[:], cos_sb[:, sl])
                    nc.vector.tensor_mul(t2[0:64, :], ps[64:128, :], sin_sb[0:64, sl])
                    nc.vector.tensor_mul(t2[64:128, :], ps[0:64, :], sin_sb[64:128, sl])
                    if dst3 is None:
                        nc.gpsimd.tensor_add(dst, t1[:], t2[:])
                    else:
                        r3 = lambda a: a.rearrange("p (a b) -> p a b", b=128)
                        nc.gpsimd.tensor_add(dst3, r3(t1[:]), r3(t2[:]))

                for tb in range(NTB):
                    tsl = slice(tb * 512, (tb + 1) * 512)
                    # V tiles, direct [t, hd] layout (no transpose needed)
                    for j in range(4):
                        t0 = (tb * 4 + j) * 128
                        vps = vpp.tile([128, 128], dt.float32, tag="vps", name="vps")
                        k = 0
                        for xs, ws in TERMS_V:
                            for cb in range(0, NCB, 2):
                                nc.tensor.matmul(
                                    vps[:], xv(xs)[:, cb:cb + 2, t0:t0 + 128],
                                    wv_(ws, HD)[:, cb:cb + 2, :],
                                    start=(k == 0), stop=(k == 23), perf_mode=DRow)
                                k += 1
                        nc.any.tensor_copy(V_sb[:, t0:t0 + 128], vps[:])
                    # K tile
                    kps = pps.tile([128, 512], dt.float32, tag="ps", name="kps")
                    k = 0
                    for xs, ws in TERMS_K:
                        for cb in range(0, NCB, 2):
                            nc.tensor.matmul(
                                kps[:], wv_(ws, HD)[:, cb:cb + 2, :],
                                xv(xs)[:, cb:cb + 2, tsl],
                                start=(k == 0), stop=(k == 23), perf_mode=DRow)
                            k += 1
                    rope_evict(kps, KT_sb[:, tsl], cosk_sb, sink_sb, tb)
                    # Q tiles (4 heads), evicted into the per-qb 4-head layout
                    for h in range(HPG):
                        qps = pps.tile([128, 512], dt.float32, tag="ps", name="qps")
                        k = 0
                        for xs, ws in TERMS_Q:
                            for cb in range(0, NCB, 2):
                                nc.tensor.matmul(
                                    qps[:], wv_(ws, HPG * HD)[:, cb:cb + 2, h * HD:(h + 1) * HD],
                                    xv(xs)[:, cb:cb + 2, tsl],
                                    start=(k == 0), stop=(k == 23), perf_mode=DRow)
                                k += 1
                        dst3 = QT4_sb[:].rearrange("p (q s) -> p q s", s=HPG * 128)[
                            :, 4 * tb:4 * tb + 4, h * 128:(h + 1) * 128]
                        rope_evict(qps, None, cosq_sb, sinq_sb, tb, dst3=dst3)

            # ---------------- attention + Wo ----------------
            with tc.tile_pool(name="st_ps", bufs=3, space="PSUM") as stp, \
                 tc.tile_pool(name="acc_ps", bufs=2, space="PSUM") as accp, \
                 tc.tile_pool(name="wo_ps", bufs=3, space="PSUM") as wop, \
                 tc.tile_pool(name="attn_sb", bufs=6) as asb, \
                 tc.tile_pool(name="den_sb", bufs=2) as dsb, \
                 tc.tile_pool(name="yn_sb", bufs=3) as ysb, \
                 tc.tile_pool(name="out_sb", bufs=2) as osb:
                Exp = mybir.ActivationFunctionType.Exp
                Copy = mybir.ActivationFunctionType.Copy

                def emit_wo(wo_qb, yhi, ylo):
                    ostg = osb.tile([128, C], dt.bfloat16, tag="ostg", name="ostg")
                    y3 = lambda t: t[:].rearrange("p (k m) -> p k m", m=128)
                    for cb4 in range(C // 512):
                        csl = slice(cb4 * 512, (cb4 + 1) * 512)
                        wps = wop.tile([128, 512], dt.float32, tag="wps", name="wps")
                        k = 0
                        for ys, ws in ((yhi, wohi_sb), (ylo, wohi_sb), (yhi, wolo_sb)):
                            for p2 in range(2):
                                nc.tensor.matmul(
                                    wps[:], y3(ys)[:, 2 * p2:2 * p2 + 2, :],
                                    wv_(ws, C)[:, 2 * p2:2 * p2 + 2, csl],
                                    start=(k == 0), stop=(k == 5), perf_mode=DRow)
                                k += 1
                        nc.any.tensor_copy(ostg[:, csl], wps[:])
                        if wo_qb >= NQB - 2:
                            nc.sync.dma_start(out_d[wo_qb * 128:(wo_qb + 1) * 128, csl],
                                              ostg[:, csl])
                    if wo_qb < NQB - 2:
                        nc.sync.dma_start(out_d[wo_qb * 128:(wo_qb + 1) * 128, :], ostg[:])

                pend = []
                for qb in range(NQB):
                    nwin = min(qb, NWINB - 1) + 1
                    qsl = slice(qb * 512, (qb + 1) * 512)
                    # scores for all 4 heads at once, one matmul per key block
                    sts = []
                    for i in range(nwin):
                        jb = qb - nwin + 1 + i
                        st = stp.tile([128, 512], dt.float32, tag="st", name="st")
                        nc.tensor.matmul(st[:], KT_sb[:, jb * 128:(jb + 1) * 128],
                                         QT4_sb[:, qsl], start=True, stop=True)
                        sts.append(st)
                    # Wo of the previous block: dense PE work that fills the
                    # exp/mask latency window of this block's attention chain.
                    if pend:
                        emit_wo(*pend.pop(0))
                    # exp + edge masks
                    pms = []
                    for i in range(nwin):
                        m = i + NWINB - nwin
                        pexp = asb.tile([128, 512], dt.bfloat16, tag="pexp", name="pexp")
                        nc.scalar.activation(pexp[:], sts[i][:], Exp)
                        if m == 0:
                            pm = asb.tile([128, 512], dt.bfloat16, tag="pmask")
                            nc.gpsimd.tensor_mul(pm[:], pexp[:], maskT4_sb[:, 0:512])
                            pms.append(pm)
                        elif m == NWINB - 1:
                            pm = asb.tile([128, 512], dt.bfloat16, tag="pmask")
                            nc.vector.tensor_mul(pm[:], pexp[:],
                                                 maskT4_sb[:, (NWINB - 1) * 512:NWINB * 512])
                            pms.append(pm)
                        else:
                            pms.append(pexp)
                    # PV accumulation (4 heads batched)
                    acc = accp.tile([128, 512], dt.float32, tag="acc", name="acc")
                    for i in range(nwin):
                        jb = qb - nwin + 1 + i
                        nc.tensor.matmul(acc[:], V_sb[:, jb * 128:(jb + 1) * 128], pms[i][:],
                                         start=(i == 0), stop=(i == nwin - 1))
                    # softmax denominator: tree adds (DVE) + partition reduce
                    work = [p[:] for p in pms]
                    while len(work) > 1:
                        nxt = []
                        for a, b in zip(work[0::2], work[1::2]):
                            t = asb.tile([128, 512], dt.bfloat16, tag="padd", name="padd")
                            nc.vector.tensor_add(t[:], a, b)
                            nxt.append(t[:])
                        if len(work) % 2:
                            nxt.append(work[-1])
                        work = nxt
                    sbc = dsb.tile([128, 512], dt.float32, tag="sbc")
                    nc.gpsimd.partition_all_reduce(sbc[:], work[0], channels=128,
                                                   reduce_op=bass_isa.ReduceOp.add)
                    rbc = dsb.tile([128, 512], dt.bfloat16, tag="rbc")
                    with nc.allow_low_precision("softmax denominator reciprocal; 2e-2 rel-err budget"):
                        nc.vector.reciprocal(rbc[:], sbc[:])
                    # ynT = acc * rbc, written as an fp8 hi/lo pair for Wo
                    t32 = ysb.tile([128, 512], dt.float32, tag="t32", name="t32")
                    nc.vector.tensor_mul(t32[:], acc[:], rbc[:])
                    yhi = ysb.tile([128, 512], f8, tag="yhi", name="yhi")
                    nc.scalar.activation(yhi[:], t32[:], Copy)
                    ylo = ysb.tile([128, 512], f8, tag="ylo", name="ylo")
                    with nc.allow_low_precision("fp8 lo residual of ynT pair"):
                        nc.vector.tensor_sub(ylo[:], t32[:], yhi[:])
                    pend.append((qb, yhi, ylo))
                while pend:
                    emit_wo(*pend.pop(0))

    nc.compile()
    return nc


def _get_nc(t_len=T):
    if t_len not in _NC_CACHE:
        _NC_CACHE[t_len] = build_nc(t_len)
    return _NC_CACHE[t_len]


def host_inputs(x, Wq, Wk, Wv, Wo, t_len=T):
    """Per-core input shards (8 dicts)."""
    x = np.asarray(x, np.float32)
    Wq = np.asarray(Wq, np.float32) * WSC
    Wk = np.asarray(Wk, np.float32) * WSC
    Wv = np.asarray(Wv, np.float32) * WSC
    Wo = np.asarray(Wo, np.float32) * WSC
    cosT, sin_swap = _rope_tables(t_len)
    common = {
        "cosq": (cosT * (SCALE / WSC)).astype(BF16),
        "sinq": (sin_swap * (SCALE / WSC)).astype(BF16),
        "cosk": (cosT / WSC).astype(BF16),
        "sink": (sin_swap / WSC).astype(BF16),
        "maskT4": _band_maskT4().astype(BF16),
    }
    in_maps = []
    for core in range(NCORES):
        b, hg = core // TPG, core % TPG
        m = dict(common)
        m["xhi"], m["xlo"] = _split_f8(np.ascontiguousarray(x[b, :t_len, :].T))
        m["wqhi"], m["wqlo"] = _split_f8(Wq[:, hg * HPG * HD:(hg + 1) * HPG * HD])
        m["wkhi"], m["wklo"] = _split_f8(Wk[:, hg * HD:(hg + 1) * HD])
        m["wvhi"], m["wvlo"] = _split_f8(Wv[:, hg * HD:(hg + 1) * HD])
        m["wohi"], m["wolo"] = _split_f8(Wo[hg * HPG * HD:(hg + 1) * HPG * HD, :])
        in_maps.append(m)
    return in_maps


def kernel(x, Wq, Wk, Wv, Wo):
    from concourse import bass_utils

    nc = _get_nc(T)
    in_maps = host_inputs(x, Wq, Wk, Wv, Wo, T)
    res = bass_utils.run_bass_kernel_spmd(nc, in_maps, core_ids=list(range(NCORES)))
    out = np.zeros((B, T, C), np.float32)
    for core in range(NCORES):
        out[core // TPG] += res.results[core]["out"].astype(np.float32)
    out *= 1.0 / (WSC * WSC)
    return out


def core_reference(x_b, Wq, Wk, Wv, Wo, hg, t_len=T):
    """Numpy reference of one core's partial output (f32 math, for dev tests)."""
    xb = np.asarray(x_b, np.float64)[:t_len]
    q = xb @ np.float64(Wq[:, hg * HPG * HD:(hg + 1) * HPG * HD])    # [T, 512]
    k = xb @ np.float64(Wk[:, hg * HD:(hg + 1) * HD])                # [T, 128]
    v = xb @ np.float64(Wv[:, hg * HD:(hg + 1) * HD])
    cosT, sin_swap = _rope_tables(t_len)
    cos = cosT.T.astype(np.float64)
    sinsw = sin_swap.T.astype(np.float64)

    def rope(z):
        zsw = np.concatenate([z[:, HD // 2:], z[:, :HD // 2]], axis=1)
        sgn = np.concatenate([sinsw[:, :HD // 2], sinsw[:, HD // 2:]], axis=1)
        return z * cos + zsw * sgn

    out = np.zeros((t_len, C), np.float64)
    i = np.arange(t_len)[:, None]
    j = np.arange(t_len)[None, :]
    allowed = (j <= i) & (i - j < WIN)
    kr = rope(k)
    for h in range(HPG):
        qh = rope(q[:, h * HD:(h + 1) * HD]) * SCALE
        s = qh @ kr.T
        s = np.where(allowed, s, -np.inf)
        p = np.exp(s - s.max(axis=1, keepdims=True))
        p /= p.sum(axis=1, keepdims=True)
        y = p @ v
        out += y @ np.float64(Wo[hg * HPG * HD + h * HD: hg * HPG * HD + (h + 1) * HD, :])
    return out.astype(np.float32)


# revision 10
# speedup vs baseline: 1.1883x; 1.0094x over previous
"""Trainium2 Bass kernel: causal sliding-window GQA self-attention.

Problem: B=2, T=2048, C=2048, 16 q-heads / 4 kv-heads, head_dim=128,
RoPE, sliding window 512, projections Wq/Wk/Wv/Wo.

Sharding: 8 cores = DP(batch=2) x TP(head-groups=4).  Core c handles
batch c//4 and q-heads [4*(c%4), 4*(c%4)+4) (one kv head c%4).  Each
core computes a partial output contribution [T, C]; the host sums the
4 head-group partials per batch (and divides by the 64^2 weight
pre-scale, see below).

Per-core kernel:
  - Projections and the Wo matmul run in fp8-e4m3 DoubleRow perf mode
    (two contraction rows per PE pass, 0.5 cycles/output-column: 4x the
    bf16 MAC rate).  Full precision is recovered with a 3-term hi-lo
    split: x ~ x_hi + x_lo (fp8 pair, host-prepared), W ~ W_hi + W_lo,
    and x@W ~ xhi@Whi + xlo@Whi + xhi@Wlo -- 24 DoubleRow matmuls per
    2048-contraction tile vs 16 bf16 matmuls, i.e. 0.75x the PE time
    with accuracy slightly better than a bf16 matmul.  Weights are
    pre-scaled by 64 on the host so W values (~0.02) sit in the fp8
    normal range; the 1/64 is folded into the RoPE eviction tables for
    Q/K, carried harmlessly through V->y->Wo for the rest, and divided
    out on the host (out = psum / 4096).
  - Attention stays bf16 (contraction is only 128 there, DoubleRow
    buys nothing at equal accuracy) but batches the 4 GQA q-heads of
    the shared kv-head into one free-dim-512 stream: per (128-query
    block, 128-key block) ONE score matmul [keys, 4*128] and ONE PV
    accumulation into ynT [hd, 4*128], quartering instruction counts.
  - exp on ScalarE (no max subtraction; max |score| ~5.5 on this input
    distribution), band-mask multiplies only on the two edge key
    blocks (leading edge on GpSimd, diagonal on DVE), softmax
    denominators via DVE pairwise adds + one GpSimd
    partition_all_reduce, reciprocal on DVE.
  - ynT is written as an fp8 hi/lo pair (DVE mul + ScalarE cast + DVE
    sub) feeding the 3-term DoubleRow Wo.
  - PE program order per query block: scores(qb) -> Wo(qb-1) -> PV(qb),
    so the late-emitted Wo matmuls fill the exp/mask latency window of
    the current block's attention chain.
  - x streams tb-major (512 t-columns at a time across all 16
    contraction row-blocks) so the first projection tile is gated on
    ~2.5MB of DMA instead of the whole 8MB x load.
"""

import os
import sys

for _p in ("/opt/trn_rl_repo", "/root/.axon_site/_ro/trn_rl_repo"):
    if os.path.isdir(_p) and _p not in sys.path:
        sys.path.append(_p)

import numpy as np
import ml_dtypes

BF16 = ml_dtypes.bfloat16
F8 = ml_dtypes.float8_e4m3fn

B, T, C = 2, 2048, 2048
H, KVH, HD = 16, 4, 128
WIN = 512
ROPE_BASE = 10000.0
NCORES = 8
TPG = 4           # tensor-parallel group count (head groups)
HPG = H // TPG    # q-heads per core
SCALE = 1.0 / float(np.sqrt(np.float32(HD)))
NWINB = WIN // 128 + 1   # 5 key blocks cover the 640-wide window
NCB = C // 128           # contraction row-blocks for projections
WSC = 64.0               # host weight pre-scale (fp8 subnormal dodge)

_NC_CACHE = {}


def _rope_tables(t_len):
    # Match reference: angles computed in float32.
    inv = (1.0 / (np.float32(ROPE_BASE) ** (np.arange(0, HD, 2, dtype=np.float32) / np.float32(HD)))).astype(np.float32)
    ang = np.arange(t_len, dtype=np.float32)[None, :] * inv[:, None]   # [64, T]
    cosT = np.concatenate([np.cos(ang), np.cos(ang)], axis=0)          # [128, T]
    sinT = np.sin(ang)
    sin_swap = np.concatenate([-sinT, sinT], axis=0)                   # [128, T]
    return cosT.astype(np.float32), sin_swap.astype(np.float32)


def _band_maskT4():
    # maskT[c, r] = 1 iff query row r may attend key col c of the
    # 640-wide window (c = j - (qs - 512)):  r+1 <= c <= r+512.
    # Replicated x4 along the free dim for the 4-head-batched layout.
    r = np.arange(128)[None, :]
    c = np.arange(NWINB * 128)[:, None]
    m = ((r + 1 <= c) & (c <= r + WIN)).astype(np.float32)             # [640, 128]
    return np.tile(m, (1, HPG))                                        # [640, 512]


def _split_f8(a):
    """fp8 hi/lo pair: a ~ hi + lo with ~7-bit effective mantissa."""
    a32 = np.asarray(a, np.float32)
    hi = a32.astype(F8)
    lo = (a32 - hi.astype(np.float32)).astype(F8)
    return hi, lo


def build_nc(t_len=T):
    """Build + compile the per-core Bass module (SPMD, identical on all cores)."""
    import concourse.mybir as mybir
    import concourse.tile as tile
    from concourse import bacc
    from concourse import bass_isa

    dt = mybir.dt
    DRow = mybir.MatmulPerfMode.DoubleRow
    NQB = t_len // 128        # query/key blocks
    NTB = t_len // 512        # 512-wide t-blocks for projections

    nc = bacc.Bacc("TRN2", target_bir_lowering=False, debug=False, num_devices=NCORES)

    def din(name, shape, d=dt.float8e4):
        return nc.dram_tensor(name, shape, d, kind="ExternalInput").ap()

    xhi_d = din("xhi", [C, t_len])
    xlo_d = din("xlo", [C, t_len])
    wqhi_d = din("wqhi", [C, HPG * HD])
    wqlo_d = din("wqlo", [C, HPG * HD])
    wkhi_d = din("wkhi", [C, HD])
    wklo_d = din("wklo", [C, HD])
    wvhi_d = din("wvhi", [C, HD])
    wvlo_d = din("wvlo", [C, HD])
    wohi_d = din("wohi", [HPG * HD, C])
    wolo_d = din("wolo", [HPG * HD, C])
    cosq_d = din("cosq", [HD, t_len], dt.bfloat16)
    sinq_d = din("sinq", [HD, t_len], dt.bfloat16)
    cosk_d = din("cosk", [HD, t_len], dt.bfloat16)
    sink_d = din("sink", [HD, t_len], dt.bfloat16)
    maskT4_d = din("maskT4", [NWINB * 128, HPG * 128], dt.bfloat16)
    out_d = nc.dram_tensor("out", [t_len, C], dt.bfloat16, kind="ExternalOutput").ap()

    with tile.TileContext(nc) as tc:
        with tc.tile_pool(name="persist", bufs=1) as pp:
            f8 = dt.float8e4
            xhi_sb = pp.tile([128, NCB * t_len], f8, tag="xhi")
            xlo_sb = pp.tile([128, NCB * t_len], f8, tag="xlo")
            wqhi_sb = pp.tile([128, NCB * HPG * HD], f8, tag="wqhi")
            wqlo_sb = pp.tile([128, NCB * HPG * HD], f8, tag="wqlo")
            wkhi_sb = pp.tile([128, NCB * HD], f8, tag="wkhi")
            wklo_sb = pp.tile([128, NCB * HD], f8, tag="wklo")
            wvhi_sb = pp.tile([128, NCB * HD], f8, tag="wvhi")
            wvlo_sb = pp.tile([128, NCB * HD], f8, tag="wvlo")
            wohi_sb = pp.tile([128, HPG * C], f8, tag="wohi")
            wolo_sb = pp.tile([128, HPG * C], f8, tag="wolo")
            cosq_sb = pp.tile([128, t_len], dt.bfloat16, tag="cosq")
            sinq_sb = pp.tile([128, t_len], dt.bfloat16, tag="sinq")
            cosk_sb = pp.tile([128, t_len], dt.bfloat16, tag="cosk")
            sink_sb = pp.tile([128, t_len], dt.bfloat16, tag="sink")
            maskT4_sb = pp.tile([128, NWINB * HPG * 128], dt.bfloat16, tag="maskT4")
            QT4_sb = pp.tile([128, NQB * HPG * 128], dt.bfloat16, tag="QT4")
            KT_sb = pp.tile([128, t_len], dt.bfloat16, tag="KT")
            V_sb = pp.tile([128, t_len], dt.bfloat16, tag="V")

            # 3D chunk views for DoubleRow operand pairing.
            def xv(ts):
                return ts[:].rearrange("p (c t) -> p c t", t=t_len)

            def wv_(ts, m):
                return ts[:].rearrange("p (c m) -> p c m", m=m)

            # ---- DMA emission order: first consumers first; x streams
            # tb-major so the first projection tile gates on ~2.5MB.
            def xload(xs, xd, tsl):
                for cb in range(0, NCB, 4):
                    nc.sync.dma_start(
                        xv(xs)[:, cb:cb + 4, tsl],
                        xd[cb * 128:(cb + 4) * 128, tsl].rearrange("(c p) t -> p c t", p=128))

            # First projection tile gates on: wv_hi + x_hi(tb0) + wv_lo +
            # x_lo(tb0) -- emit exactly in that order so the PE's first hi*hi
            # matmuls start as early as possible.
            nc.sync.dma_start(wv_(wvhi_sb, HD), wvhi_d.rearrange("(c p) h -> p c h", p=128))
            for tb in range(NTB):
                tsl = slice(tb * 512, (tb + 1) * 512)
                if tb == 0:
                    xload(xhi_sb, xhi_d, tsl)
                    nc.sync.dma_start(wv_(wvlo_sb, HD), wvlo_d.rearrange("(c p) h -> p c h", p=128))
                    xload(xlo_sb, xlo_d, tsl)
                    nc.sync.dma_start(wv_(wkhi_sb, HD), wkhi_d.rearrange("(c p) h -> p c h", p=128))
                    nc.sync.dma_start(wv_(wklo_sb, HD), wklo_d.rearrange("(c p) h -> p c h", p=128))
                else:
                    xload(xhi_sb, xhi_d, tsl)
                    xload(xlo_sb, xlo_d, tsl)
                if tb == 0:
                    nc.sync.dma_start(wv_(wqhi_sb, HPG * HD), wqhi_d.rearrange("(c p) m -> p c m", p=128))
                    nc.sync.dma_start(wv_(wqlo_sb, HPG * HD), wqlo_d.rearrange("(c p) m -> p c m", p=128))
                    nc.sync.dma_start(cosk_sb[:], cosk_d)
                    nc.sync.dma_start(sink_sb[:], sink_d)
                    nc.sync.dma_start(cosq_sb[:], cosq_d)
                    nc.sync.dma_start(sinq_sb[:], sinq_d)
                if tb == 1:
                    nc.sync.dma_start(maskT4_sb[:].rearrange("p (m c) -> p m c", c=HPG * 128),
                                      maskT4_d.rearrange("(m p) c -> p m c", p=128))
                    nc.sync.dma_start(wv_(wohi_sb, C), wohi_d.rearrange("(h p) c -> p h c", p=128))
                    nc.sync.dma_start(wv_(wolo_sb, C), wolo_d.rearrange("(h p) c -> p h c", p=128))

            TERMS_V = ((xhi_sb, wvhi_sb), (xlo_sb, wvhi_sb), (xhi_sb, wvlo_sb))
            TERMS_K = ((xhi_sb, wkhi_sb), (xlo_sb, wkhi_sb), (xhi_sb, wklo_sb))
            TERMS_Q = ((xhi_sb, wqhi_sb), (xlo_sb, wqhi_sb), (xhi_sb, wqlo_sb))

            # ---------------- projections ----------------
            with tc.tile_pool(name="proj_ps", bufs=5, space="PSUM") as pps, \
                 tc.tile_pool(name="v_ps", bufs=2, space="PSUM") as vpp, \
                 tc.tile_pool(name="rope_scr", bufs=3) as rsc:

                def rope_evict(ps, dst, cos_sb, sin_sb, tb, dst3=None):
                    sl = slice(tb * 512, (tb + 1) * 512)
                    t1 = rsc.tile([128, 512], dt.float32, tag="t1")
                    t2 = rsc.tile([128, 512], dt.float32, tag="t2")
                    nc.vector.tensor_mul(t1[:], ps[:], cos_sb[:, sl])
                    nc.vector.tensor_mul(t2[0:64, :], ps[64:128, :], sin_sb[0:64, sl])
                    nc.vector.tensor_mul(t2[64:128, :], ps[0:64, :], sin_sb[64:128, sl])
                    if dst3 is None:
                        nc.gpsimd.tensor_add(dst, t1[:], t2[:])
                    else:
                        r3 = lambda a: a.rearrange("p (a b) -> p a b", b=128)
                        nc.gpsimd.tensor_add(dst3, r3(t1[:]), r3(t2[:]))

                for tb in range(NTB):
                    tsl = slice(tb * 512, (tb + 1) * 512)
                    # V tiles, direct [t, hd] layout (no transpose needed)
                    for j in range(4):
                        t0 = (tb * 4 + j) * 128
                        vps = vpp.tile([128, 128], dt.float32, tag="vps", name="vps")
                        k = 0
                        for xs, ws in TERMS_V:
                            for cb in range(0, NCB, 2):
                                nc.tensor.matmul(
                                    vps[:], xv(xs)[:, cb:cb + 2, t0:t0 + 128],
                                    wv_(ws, HD)[:, cb:cb + 2, :],
                                    start=(k == 0), stop=(k == 23), perf_mode=DRow)
                                k += 1
                        nc.any.tensor_copy(V_sb[:, t0:t0 + 128], vps[:])
                    # K tile
                    kps = pps.tile([128, 512], dt.float32, tag="ps", name="kps")
                    k = 0
                    for xs, ws in TERMS_K:
                        for cb in range(0, NCB, 2):
                            nc.tensor.matmul(
                                kps[:], wv_(ws, HD)[:, cb:cb + 2, :],
                                xv(xs)[:, cb:cb + 2, tsl],
                                start=(k == 0), stop=(k == 23), perf_mode=DRow)
                            k += 1
                    rope_evict(kps, KT_sb[:, tsl], cosk_sb, sink_sb, tb)
                    # Q tiles (4 heads), evicted into the per-qb 4-head layout
                    for h in range(HPG):
                        qps = pps.tile([128, 512], dt.float32, tag="ps", name="qps")
                        k = 0
                        for xs, ws in TERMS_Q:
                            for cb in range(0, NCB, 2):
                                nc.tensor.matmul(
                                    qps[:], wv_(ws, HPG * HD)[:, cb:cb + 2, h * HD:(h + 1) * HD],
                                    xv(xs)[:, cb:cb + 2, tsl],
                                    start=(k == 0), stop=(k == 23), perf_mode=DRow)
                                k += 1
                        dst3 = QT4_sb[:].rearrange("p (q s) -> p q s", s=HPG * 128)[
                            :, 4 * tb:4 * tb + 4, h * 128:(h + 1) * 128]
                        rope_evict(qps, None, cosq_sb, sinq_sb, tb, dst3=dst3)

            # ---------------- attention + Wo ----------------
            with tc.tile_pool(name="st_ps", bufs=4, space="PSUM") as stp, \
                 tc.tile_pool(name="acc_ps", bufs=2, space="PSUM") as accp, \
                 tc.tile_pool(name="wo_ps", bufs=2, space="PSUM") as wop, \
                 tc.tile_pool(name="attn_sb", bufs=6) as asb, \
                 tc.tile_pool(name="den_sb", bufs=2) as dsb, \
                 tc.tile_pool(name="yn_sb", bufs=3) as ysb, \
                 tc.tile_pool(name="out_sb", bufs=2) as osb:
                Exp = mybir.ActivationFunctionType.Exp
                Copy = mybir.ActivationFunctionType.Copy

                def emit_wo(wo_qb, yhi, ylo):
                    ostg = osb.tile([128, C], dt.bfloat16, tag="ostg", name="ostg")
                    y3 = lambda t: t[:].rearrange("p (k m) -> p k m", m=128)
                    for cb4 in range(C // 512):
                        csl = slice(cb4 * 512, (cb4 + 1) * 512)
                        wps = wop.tile([128, 512], dt.float32, tag="wps", name="wps")
                        k = 0
                        for ys, ws in ((yhi, wohi_sb), (ylo, wohi_sb), (yhi, wolo_sb)):
                            for p2 in range(2):
                                nc.tensor.matmul(
                                    wps[:], y3(ys)[:, 2 * p2:2 * p2 + 2, :],
                                    wv_(ws, C)[:, 2 * p2:2 * p2 + 2, csl],
                                    start=(k == 0), stop=(k == 5), perf_mode=DRow)
                                k += 1
                        # eviction engine split: DVE is near-critical, Pool has
                        # slack -- 1 copy on DVE, 3 on Pool.
                        if cb4 == 0:
                            nc.vector.tensor_copy(ostg[:, csl], wps[:])
                        else:
                            nc.gpsimd.tensor_copy(ostg[:, csl], wps[:])
                        if wo_qb >= NQB - 2:
                            nc.sync.dma_start(out_d[wo_qb * 128:(wo_qb + 1) * 128, csl],
                                              ostg[:, csl])
                    if wo_qb < NQB - 2:
                        nc.sync.dma_start(out_d[wo_qb * 128:(wo_qb + 1) * 128, :], ostg[:])

                pend = []
                for qb in range(NQB):
                    nwin = min(qb, NWINB - 1) + 1
                    qsl = slice(qb * 512, (qb + 1) * 512)
                    # scores for all 4 heads at once, one matmul per key block
                    sts = []
                    for i in range(nwin):
                        jb = qb - nwin + 1 + i
                        st = stp.tile([128, 512], dt.float32, tag="st", name="st")
                        nc.tensor.matmul(st[:], KT_sb[:, jb * 128:(jb + 1) * 128],
                                         QT4_sb[:, qsl], start=True, stop=True)
                        sts.append(st)
                    # Wo of the previous block: dense PE work that fills the
                    # exp/mask latency window of this block's attention chain.
                    if pend:
                        emit_wo(*pend.pop(0))
                    # exp + edge masks
                    pms = []
                    for i in range(nwin):
                        m = i + NWINB - nwin
                        pexp = asb.tile([128, 512], dt.bfloat16, tag="pexp", name="pexp")
                        nc.scalar.activation(pexp[:], sts[i][:], Exp)
                        if m == 0:
                            pm = asb.tile([128, 512], dt.bfloat16, tag="pmask")
                            nc.gpsimd.tensor_mul(pm[:], pexp[:], maskT4_sb[:, 0:512])
                            pms.append(pm)
                        elif m == NWINB - 1:
                            pm = asb.tile([128, 512], dt.bfloat16, tag="pmask")
                            nc.vector.tensor_mul(pm[:], pexp[:],
                                                 maskT4_sb[:, (NWINB - 1) * 512:NWINB * 512])
                            pms.append(pm)
                        else:
                            pms.append(pexp)
                    # PV accumulation (4 heads batched)
                    acc = accp.tile([128, 512], dt.float32, tag="acc", name="acc")
                    for i in range(nwin):
                        jb = qb - nwin + 1 + i
                        nc.tensor.matmul(acc[:], V_sb[:, jb * 128:(jb + 1) * 128], pms[i][:],
                                         start=(i == 0), stop=(i == nwin - 1))
                    # softmax denominator: tree adds (DVE) + partition reduce
                    work = [p[:] for p in pms]
                    while len(work) > 1:
                        nxt = []
                        for a, b in zip(work[0::2], work[1::2]):
                            t = asb.tile([128, 512], dt.bfloat16, tag="padd", name="padd")
                            nc.vector.tensor_add(t[:], a, b)
                            nxt.append(t[:])
                        if len(work) % 2:
                            nxt.append(work[-1])
                        work = nxt
                    sbc = dsb.tile([128, 512], dt.float32, tag="sbc")
                    nc.gpsimd.partition_all_reduce(sbc[:], work[0], channels=128,
                                                   reduce_op=bass_isa.ReduceOp.add)
                    rbc = dsb.tile([128, 512], dt.bfloat16, tag="rbc")
                    with nc.allow_low_precision("softmax denominator reciprocal; 2e-2 rel-err budget"):
                        nc.vector.reciprocal(rbc[:], sbc[:])
                    # ynT = acc * rbc, written as an fp8 hi/lo pair for Wo
                    t32 = ysb.tile([128, 512], dt.float32, tag="t32", name="t32")
                    nc.vector.tensor_mul(t32[:], acc[:], rbc[:])
                    yhi = ysb.tile([128, 512], f8, tag="yhi", name="yhi")
                    nc.vector.tensor_copy(yhi[:], t32[:])
                    ylo = ysb.tile([128, 512], f8, tag="ylo", name="ylo")
                    with nc.allow_low_precision("fp8 lo residual of ynT pair"):
                        nc.vector.tensor_sub(ylo[:], t32[:], yhi[:])
                    pend.append((qb, yhi, ylo))
                while pend:
                    emit_wo(*pend.pop(0))

    nc.compile()
    return nc


def _get_nc(t_len=T):
    if t_len not in _NC_CACHE:
        _NC_CACHE[t_len] = build_nc(t_len)
    return _NC_CACHE[t_len]


def host_inputs(x, Wq, Wk, Wv, Wo, t_len=T):
    """Per-core input shards (8 dicts)."""
    x = np.asarray(x, np.float32)
    Wq = np.asarray(Wq, np.float32) * WSC
    Wk = np.asarray(Wk, np.float32) * WSC
    Wv = np.asarray(Wv, np.float32) * WSC
    Wo = np.asarray(Wo, np.float32) * WSC
    cosT, sin_swap = _rope_tables(t_len)
    common = {
        "cosq": (cosT * (SCALE / WSC)).astype(BF16),
        "sinq": (sin_swap * (SCALE / WSC)).astype(BF16),
        "cosk": (cosT / WSC).astype(BF16),
        "sink": (sin_swap / WSC).astype(BF16),
        "maskT4": _band_maskT4().astype(BF16),
    }
    in_maps = []
    for core in range(NCORES):
        b, hg = core // TPG, core % TPG
        m = dict(common)
        m["xhi"], m["xlo"] = _split_f8(np.ascontiguousarray(x[b, :t_len, :].T))
        m["wqhi"], m["wqlo"] = _split_f8(Wq[:, hg * HPG * HD:(hg + 1) * HPG * HD])
        m["wkhi"], m["wklo"] = _split_f8(Wk[:, hg * HD:(hg + 1) * HD])
        m["wvhi"], m["wvlo"] = _split_f8(Wv[:, hg * HD:(hg + 1) * HD])
        m["wohi"], m["wolo"] = _split_f8(Wo[hg * HPG * HD:(hg + 1) * HPG * HD, :])
        in_maps.append(m)
    return in_maps


def kernel(x, Wq, Wk, Wv, Wo):
    from concourse import bass_utils

    nc = _get_nc(T)
    in_maps = host_inputs(x, Wq, Wk, Wv, Wo, T)
    res = bass_utils.run_bass_kernel_spmd(nc, in_maps, core_ids=list(range(NCORES)))
    out = np.zeros((B, T, C), np.float32)
    for core in range(NCORES):
        out[core // TPG] += res.results[core]["out"].astype(np.float32)
    out *= 1.0 / (WSC * WSC)
    return out


def core_reference(x_b, Wq, Wk, Wv, Wo, hg, t_len=T):
    """Numpy reference of one core's partial output (f32 math, for dev tests)."""
    xb = np.asarray(x_b, np.float64)[:t_len]
    q = xb @ np.float64(Wq[:, hg * HPG * HD:(hg + 1) * HPG * HD])    # [T, 512]
    k = xb @ np.float64(Wk[:, hg * HD:(hg + 1) * HD])                # [T, 128]
    v = xb @ np.float64(Wv[:, hg * HD:(hg + 1) * HD])
    cosT, sin_swap = _rope_tables(t_len)
    cos = cosT.T.astype(np.float64)
    sinsw = sin_swap.T.astype(np.float64)

    def rope(z):
        zsw = np.concatenate([z[:, HD // 2:], z[:, :HD // 2]], axis=1)
        sgn = np.concatenate([sinsw[:, :HD // 2], sinsw[:, HD // 2:]], axis=1)
        return z * cos + zsw * sgn

    out = np.zeros((t_len, C), np.float64)
    i = np.arange(t_len)[:, None]
    j = np.arange(t_len)[None, :]
    allowed = (j <= i) & (i - j < WIN)
    kr = rope(k)
    for h in range(HPG):
        qh = rope(q[:, h * HD:(h + 1) * HD]) * SCALE
        s = qh @ kr.T
        s = np.where(allowed, s, -np.inf)
        p = np.exp(s - s.max(axis=1, keepdims=True))
        p /= p.sum(axis=1, keepdims=True)
        y = p @ v
        out += y @ np.float64(Wo[hg * HPG * HD + h * HD: hg * HPG * HD + (h + 1) * HD, :])
    return out.astype(np.float32)
